# revision 1
# baseline (speedup 1.0000x reference)
"""Multi-Head Latent Attention (MLA) Bass kernel for 8 trn2 NeuronCores.

Sharding: core c handles batch b=c//4 and head group hg=c%4 (4 of 16 heads).
Host transposes x[b] once and pre-casts everything to bf16; the device
pipeline runs in "transposed" layout (feature dims on SBUF partitions).

v3 design (instruction-count-minimized; this platform costs ~0.5us per
matmul instruction regardless of N, so MM count is king):
  - q-path FOLDED on host: with only 4 of 16 heads per core, computing the
    shared compression c_q then up-projecting costs more instructions than
    direct projection with A_h = W_dq @ W_uq_h (and W_dq @ W_qr_p). This
    cuts 672 MM instructions per body vs the two-stage path.
  - kv-path keeps the two-stage form (KV=512 compression amortizes over
    k/v/rope for 4 heads).
  - single pass per 512-token block: projections -> causal attention
    (exact 128-granular trims, in-place diagonal mask) -> W_o -> out DMA.
  - all operands bf16 (matmul 1 cycle/row), fp32 PSUM accumulation,
    everything SBUF-resident, no DRAM round trips.
"""

import numpy as np

T = 2048
C = 2048
QC = 1536
KV = 512
NH = 16
DH = 128
R = 64
TB = 512           # time block / q-group width
NTB = T // TB      # 4
SCALE = 1.0 / float(np.sqrt(DH + R))
ROPE_BASE = 10000.0

_CACHE = {}

# Ablation hooks used during development; always full kernel for grading.
ABLATE = ""


def _build_nc(repeat=1):
    import concourse.bacc as bacc
    import concourse.mybir as mybir
    import concourse.tile as tile

    BF16 = mybir.dt.bfloat16
    F32 = mybir.dt.float32

    nc = bacc.Bacc("TRN2", target_bir_lowering=False, debug=False)

    xT = nc.dram_tensor("xT", [C, T], BF16, kind="ExternalInput")
    aq = nc.dram_tensor("aq", [C, 512], BF16, kind="ExternalInput")
    aqr = nc.dram_tensor("aqr", [C, 256], BF16, kind="ExternalInput")
    wdkv = nc.dram_tensor("wdkv", [C, KV], BF16, kind="ExternalInput")
    wuk = nc.dram_tensor("wuk", [KV, 512], BF16, kind="ExternalInput")
    wuv = nc.dram_tensor("wuv", [KV, 512], BF16, kind="ExternalInput")
    wkr = nc.dram_tensor("wkr", [KV, 256], BF16, kind="ExternalInput")
    wo = nc.dram_tensor("wo", [512, C], BF16, kind="ExternalInput")
    cosd = nc.dram_tensor("cosd", [128, T], F32, kind="ExternalInput")
    sind = nc.dram_tensor("sind", [128, T], F32, kind="ExternalInput")
    maskd = nc.dram_tensor("maskd", [128, 128], F32, kind="ExternalInput")
    onesd = nc.dram_tensor("onesd", [128, 128], BF16, kind="ExternalInput")
    out = nc.dram_tensor("out", [T, C], F32, kind="ExternalOutput")

    with tile.TileContext(nc) as tc:
        for _rep in range(repeat):
            _emit_body(nc, tc, mybir,
                       xT, aq, aqr, wdkv, wuk, wuv, wkr, wo,
                       cosd, sind, maskd, onesd, out)

    nc.compile()
    return nc


def _emit_body(nc, tc, mybir,
               xT, aq, aqr, wdkv, wuk, wuv, wkr, wo,
               cosd, sind, maskd, onesd, out):
    BF16 = mybir.dt.bfloat16
    F32 = mybir.dt.float32
    AF = mybir.ActivationFunctionType

    def ecopy(eng, dst, src_):
        (eng.copy if eng is nc.scalar else eng.tensor_copy)(dst, src_)

    with (
        tc.tile_pool(name="p1", bufs=1) as sp,
        tc.tile_pool(name="p1ps", bufs=1, space="PSUM") as pp,
    ):
        cos_sb = sp.tile([128, T], F32, name="cos_sb")
        nc.sync.dma_start(cos_sb[:], cosd[:])
        sin_sb = sp.tile([128, T], F32, name="sin_sb")
        nc.scalar.dma_start(sin_sb[:], sind[:])
        mask_sb = sp.tile([128, 128], F32, name="mask_sb")
        nc.sync.dma_start(mask_sb[:], maskd[:])
        ones_sb = sp.tile([128, 128], BF16, name="ones_sb")
        nc.scalar.dma_start(ones_sb[:], onesd[:])
        aq_sb = sp.tile([128, 16, 512], BF16, name="aq_sb")
        nc.sync.dma_start(aq_sb[:], aq.rearrange("(k p) n -> p k n", p=128))
        aqr_sb = sp.tile([128, 16, 256], BF16, name="aqr_sb")
        nc.scalar.dma_start(aqr_sb[:], aqr.rearrange("(k p) n -> p k n", p=128))
        wdkv_sb = sp.tile([128, 16, KV], BF16, name="wdkv_sb")
        nc.sync.dma_start(wdkv_sb[:], wdkv.rearrange("(k p) n -> p k n", p=128))
        wuk_sb = sp.tile([128, 4, 512], BF16, name="wuk_sb")
        nc.scalar.dma_start(wuk_sb[:], wuk.rearrange("(k p) n -> p k n", p=128))
        wuv_sb = sp.tile([128, 4, 512], BF16, name="wuv_sb")
        nc.sync.dma_start(wuv_sb[:], wuv.rearrange("(k p) n -> p k n", p=128))
        wkr_sb = sp.tile([128, 4, 256], BF16, name="wkr_sb")
        nc.scalar.dma_start(wkr_sb[:], wkr.rearrange("(k p) n -> p k n", p=128))
        wo_sb = sp.tile([128, 4, C], BF16, name="wo_sb")
        nc.sync.dma_start(wo_sb[:], wo.rearrange("(h p) n -> p h n", p=128))

        kc_t = [sp.tile([128, 4, TB], BF16, name=f"kc{t}") for t in range(NTB)]
        kr_t = [sp.tile([128, 2, TB], BF16, name=f"kr{t}") for t in range(NTB)]
        v_t = [sp.tile([128, 4, TB], BF16, name=f"v{t}") for t in range(NTB)]

        def rope_store(ps_t, dst, cs, sn):
            # ps_t [128, TB]: rows [64 head 2p | 64 head 2p+1] rope dims
            t1 = sp.tile([128, TB], BF16, name="rp1", tag="rp1", bufs=2)
            nc.vector.tensor_mul(t1[:], ps_t[:], cs)
            sh = sp.tile([128, TB], BF16, name="rp2", tag="rp2", bufs=2)
            nc.vector.tensor_copy(sh[0:32, :], ps_t[32:64, :])
            nc.vector.tensor_copy(sh[32:64, :], ps_t[0:32, :])
            nc.vector.tensor_copy(sh[64:96, :], ps_t[96:128, :])
            nc.vector.tensor_copy(sh[96:128, :], ps_t[64:96, :])
            nc.vector.tensor_mul(sh[:], sh[:], sn)
            nc.vector.tensor_add(dst, t1[:], sh[:])

        for t in range(NTB):
            tc0 = TB * t
            xblk = sp.tile([128, 16, TB], BF16, name="xblk", tag="xblk", bufs=2)
            eng = nc.sync if t % 2 == 0 else nc.scalar
            eng.dma_start(
                xblk[:], xT[:, tc0:tc0 + TB].rearrange("(k p) n -> p k n", p=128))

            # q content per head, folded from x
            qc_blk = sp.tile([128, 4, TB], BF16, name="qc_blk", tag="qc", bufs=2)
            for h in range(4):
                ps_t = pp.tile([128, TB], F32, name="ps_p", tag="ps_p", bufs=2)
                for k in range(16):
                    nc.tensor.matmul(
                        ps_t[:], aq_sb[:, k, 128 * h:128 * (h + 1)],
                        xblk[:, k, :], start=(k == 0), stop=(k == 15))
                eng = nc.scalar if h % 2 == 0 else nc.vector
                ecopy(eng, qc_blk[:, h, :], ps_t[:])
            # q rope per head-pair, folded from x
            qr_blk = sp.tile([128, 2, TB], BF16, name="qr_blk", tag="qr", bufs=2)
            for p in range(2):
                ps_t = pp.tile([128, TB], F32, name="ps_p", tag="ps_p", bufs=2)
                for k in range(16):
                    nc.tensor.matmul(
                        ps_t[:], aqr_sb[:, k, 128 * p:128 * (p + 1)],
                        xblk[:, k, :], start=(k == 0), stop=(k == 15))
                rope_store(ps_t, qr_blk[:, p, :],
                           cos_sb[:, tc0:tc0 + TB], sin_sb[:, tc0:tc0 + TB])
            # c_kv block
            ckv_blk = sp.tile([128, 4, TB], BF16, name="ckv_blk", tag="ckv", bufs=2)
            for m in range(4):
                ps_t = pp.tile([128, TB], F32, name="ps_p", tag="ps_p", bufs=2)
                for k in range(16):
                    nc.tensor.matmul(ps_t[:], wdkv_sb[:, k, 128 * m:128 * (m + 1)],
                                     xblk[:, k, :], start=(k == 0), stop=(k == 15))
                eng = nc.scalar if m % 2 == 0 else nc.vector
                ecopy(eng, ckv_blk[:, m, :], ps_t[:])
            # k content per head (resident)
            for h in range(4):
                ps_t = pp.tile([128, TB], F32, name="ps_p", tag="ps_p", bufs=2)
                for k in range(4):
                    nc.tensor.matmul(
                        ps_t[:], wuk_sb[:, k, 128 * h:128 * (h + 1)],
                        ckv_blk[:, k, :], start=(k == 0), stop=(k == 3))
                eng = nc.scalar if h % 2 == 0 else nc.vector
                ecopy(eng, kc_t[t][:, h, :], ps_t[:])
            # k rope per head-pair (resident)
            for p in range(2):
                ps_t = pp.tile([128, TB], F32, name="ps_p", tag="ps_p", bufs=2)
                for k in range(4):
                    nc.tensor.matmul(
                        ps_t[:], wkr_sb[:, k, 128 * p:128 * (p + 1)],
                        ckv_blk[:, k, :], start=(k == 0), stop=(k == 3))
                rope_store(ps_t, kr_t[t][:, p, :],
                           cos_sb[:, tc0:tc0 + TB], sin_sb[:, tc0:tc0 + TB])
            # v natural [tk, 4*dh] (resident)
            for tkc in range(4):
                ps_t = pp.tile([128, TB], F32, name="ps_p", tag="ps_p", bufs=2)
                for k in range(4):
                    nc.tensor.matmul(
                        ps_t[:], ckv_blk[:, k, 128 * tkc:128 * (tkc + 1)],
                        wuv_sb[:, k, :], start=(k == 0), stop=(k == 3))
                eng = nc.scalar if tkc % 2 == 0 else nc.vector
                ecopy(eng, v_t[t][:, tkc, :], ps_t[:])

            if ABLATE == "proj":
                continue
            # ---- attention for q block t ----
            nch = 4 * (t + 1)
            avn = sp.tile([128, 4 * TB], BF16, name="avn", tag="avn", bufs=2)
            for h in range(4):
                p0 = 64 * (h % 2)
                pr = h // 2
                ps_av = pp.tile([128, TB], F32, name="ps_av", tag="ps_av", bufs=2)
                ps_sum = pp.tile([1, TB], F32, name="ps_sum", tag="ps_m", bufs=2)

                def qk(c):
                    j = c - 4 * t
                    s = 128 * j if j > 0 else 0
                    ps_s = pp.tile([128, TB], F32, name="ps_s", tag="ps_s", bufs=2)
                    blk, jj = c // 4, c % 4
                    nc.tensor.matmul(ps_s[:, s:],
                                     kc_t[blk][:, h, 128 * jj:128 * (jj + 1)],
                                     qc_blk[:, h, s:], start=True, stop=False)
                    nc.tensor.matmul(ps_s[:, s:],
                                     kr_t[blk][p0:p0 + 64, pr, 128 * jj:128 * (jj + 1)],
                                     qr_blk[p0:p0 + 64, pr, s:], start=False, stop=True)
                    return ps_s

                qkq = [qk(0)]
                pendl = []
                sum_started = False
                for c in range(nch):
                    cur = qkq.pop(0)
                    if c + 1 < nch:
                        qkq.append(qk(c + 1))
                    j = c - 4 * t
                    s = 128 * j if j > 0 else 0
                    ex = sp.tile([128, TB], BF16, name="ex", tag="ex", bufs=6)
                    nc.scalar.activation(ex[:, s:], cur[:, s:], AF.Exp, scale=SCALE)
                    if j >= 0:
                        # in-place mask of the diagonal 128-col triangle
                        nc.vector.tensor_mul(ex[:, s:s + 128], ex[:, s:s + 128],
                                             mask_sb[:])
                    blk, jj = c // 4, c % 4
                    nc.tensor.matmul(ps_av[:, s:],
                                     v_t[blk][:, jj, 128 * h:128 * (h + 1)],
                                     ex[:, s:], start=(c == 0), stop=(c == nch - 1))
                    # denominator: pre-add full-width (non-diagonal) exp
                    # quads on DVE so each ones-matmul covers four chunks
                    # (non-diagonal count is 4t -> no leftovers)
                    if j < 0:
                        pendl.append(ex)
                        if len(pendl) == 4:
                            exs = sp.tile([128, TB], BF16, name="exs", tag="exs",
                                          bufs=2)
                            nc.vector.tensor_add(exs[:], pendl[0][:], pendl[1][:])
                            nc.vector.tensor_add(exs[:], exs[:], pendl[2][:])
                            nc.vector.tensor_add(exs[:], exs[:], pendl[3][:])
                            nc.tensor.matmul(ps_sum[:], ones_sb[:, 0:1], exs[:],
                                             start=not sum_started, stop=False)
                            sum_started = True
                            pendl = []
                    else:
                        nc.tensor.matmul(ps_sum[:, s:], ones_sb[:, 0:1], ex[:, s:],
                                         start=not sum_started,
                                         stop=(c == nch - 1))
                        sum_started = True
                recip = sp.tile([1, TB], F32, name="recip", tag="recip", bufs=2)
                nc.vector.reciprocal(recip[:], ps_sum[:])
                rec16 = sp.tile([1, TB], BF16, name="rec16", tag="rec16", bufs=2)
                nc.vector.tensor_copy(rec16[:], recip[:])
                ps_bc = pp.tile([128, TB], F32, name="ps_bc", tag="ps_m", bufs=2)
                nc.tensor.matmul(ps_bc[:], ones_sb[0:1, :], rec16[:],
                                 start=True, stop=True)
                av16 = sp.tile([128, TB], BF16, name="av16", tag="av16", bufs=2)
                nc.scalar.copy(av16[:], ps_av[:])
                nc.vector.tensor_mul(avn[:, TB * h:TB * (h + 1)], av16[:], ps_bc[:])

            if ABLATE == "nowo":
                continue
            # ---- W_o partials for block t ----
            for tqc in range(4):
                for n in range(4):
                    ps_o = pp.tile([128, 512], F32, name="ps_o", tag="ps_s", bufs=2)
                    for h in range(4):
                        nc.tensor.matmul(
                            ps_o[:],
                            avn[:, TB * h + 128 * tqc:TB * h + 128 * (tqc + 1)],
                            wo_sb[:, h, 512 * n:512 * (n + 1)],
                            start=(h == 0), stop=(h == 3))
                    ost = sp.tile([128, 512], F32, name="ost", tag="ost", bufs=2)
                    eng = nc.scalar if n % 2 == 0 else nc.vector
                    ecopy(eng, ost[:], ps_o[:])
                    deng = nc.sync if n % 2 == 0 else nc.scalar
                    deng.dma_start(
                        out[tc0 + 128 * tqc:tc0 + 128 * (tqc + 1),
                            512 * n:512 * (n + 1)], ost[:])


def _rope_tables():
    inv = 1.0 / (ROPE_BASE ** (np.arange(0, R, 2, dtype=np.float32) / R))
    freqs = np.arange(T, dtype=np.float32)[:, None] * inv[None, :]       # [T, 32]
    emb = np.concatenate([freqs, freqs], axis=-1)                         # [T, 64]
    cosT = np.ascontiguousarray(np.cos(emb).T.astype(np.float32))         # [64, T]
    sinT = np.ascontiguousarray(np.sin(emb).T.astype(np.float32))
    cosd = np.concatenate([cosT, cosT], axis=0)                           # [128, T]
    sin_sgn = np.concatenate([-sinT[0:32], sinT[32:64]], axis=0)          # [64, T]
    sind = np.concatenate([sin_sgn, sin_sgn], axis=0)
    return cosd, sind


def host_inmaps(inputs):
    import ml_dtypes
    BF = ml_dtypes.bfloat16

    x = np.asarray(inputs["x"], dtype=np.float32)
    W_dq = np.asarray(inputs["W_dq"], dtype=np.float32)
    W_uq = np.asarray(inputs["W_uq"], dtype=np.float32)
    W_qr = np.asarray(inputs["W_qr"], dtype=np.float32)
    W_dkv = np.asarray(inputs["W_dkv"], dtype=np.float32).astype(BF)
    W_uk = np.asarray(inputs["W_uk"], dtype=np.float32).astype(BF)
    W_uv = np.asarray(inputs["W_uv"], dtype=np.float32).astype(BF)
    W_kr = np.asarray(inputs["W_kr"], dtype=np.float32).astype(BF)
    W_o = np.asarray(inputs["W_o"], dtype=np.float32).astype(BF)

    Aq = (W_dq @ W_uq).astype(BF)     # [C, NH*DH] folded q-content projection
    Aqr = (W_dq @ W_qr).astype(BF)    # [C, NH*R] folded q-rope projection

    cosd, sind = _rope_tables()
    maskv = (np.arange(128)[:, None] <= np.arange(128)[None, :]).astype(np.float32)
    onesv = np.ones((128, 128), dtype=np.float32).astype(BF)

    in_maps = []
    for core in range(8):
        b, hg = core // 4, core % 4
        in_maps.append({
            "xT": np.ascontiguousarray(x[b].T).astype(BF),
            "aq": np.ascontiguousarray(Aq[:, 512 * hg:512 * (hg + 1)]),
            "aqr": np.ascontiguousarray(Aqr[:, 256 * hg:256 * (hg + 1)]),
            "wdkv": W_dkv,
            "wuk": np.ascontiguousarray(W_uk[:, 512 * hg:512 * (hg + 1)]),
            "wuv": np.ascontiguousarray(W_uv[:, 512 * hg:512 * (hg + 1)]),
            "wkr": np.ascontiguousarray(W_kr[:, 256 * hg:256 * (hg + 1)]),
            "wo": np.ascontiguousarray(W_o[512 * hg:512 * (hg + 1), :]),
            "cosd": cosd,
            "sind": sind,
            "maskd": maskv,
            "onesd": onesv,
        })
    return in_maps


def kernel(**inputs):
    from concourse.bass_utils import run_bass_kernel_spmd

    if "nc" not in _CACHE:
        _CACHE["nc"] = _build_nc()
    nc = _CACHE["nc"]

    in_maps = host_inmaps(inputs)

    res = run_bass_kernel_spmd(nc, in_maps, core_ids=list(range(8)))
    outs = [r["out"] for r in res.results]
    out0 = outs[0] + outs[1] + outs[2] + outs[3]
    out1 = outs[4] + outs[5] + outs[6] + outs[7]
    return np.stack([out0, out1]).astype(np.float32)

